# revision 15
# baseline (speedup 1.0000x reference)
"""HSIC pairwise loss kernel for trn2 (8 NeuronCores), fp8 DoubleRow.

Math: reference builds K_c = (w^2 w^2T) * (E_c E_c^T), M_c = R K_c, and sums
tr(M_i M_j) over i<j. With F_c = w^2 * E_c (row scaling), R the centering
matrix (idempotent):
    tr(R K_i R K_j) = ||G_i^T G_j||_F^2,  G_c = F_c - colmean(F_c)
and with A_ij = F_i^T F_j, s_c = F_c^T 1:
    G_i^T G_j = A_ij - (1/n) s_i s_j^T
so loss = sum_{i<j} ||A_ij - s_i s_j^T / n||_F^2 / (n-1)^2.

Device work: the 45 A_ij blocks [256,256] (contraction over n=4096) at
half-chunk granularity: 180 unordered cross-parent pairs of the 20
128-col units.  Decomposition: at the PARENT (chunk) level, cover K10's
45 edges with 8 bipartite K2,3 graphs (one per core, exactly 3 edges
double-covered).  Core c loads its 2 left parents (4 units, slots 0-3)
and 3 right parents (6 units, slots 4-9) = 10 units, 5.2 MB fp8, and
computes all 24 left x right unit blocks as six 512-wide DoubleRow
windows into 6 PSUM banks (24 blocks/core vs the 26 of the previous
window-search assignment; 22.5 is the absolute lower bound).

Input DMA rides BOTH HWDGE rings (sync/SP + scalar/ACT, concurrent
FIFO rings) - supertile halves split across rings, so supertile 0
lands ~0.7us after trigger and the stream sustains ~2x one ring.  The
PSUM drain runs on vector only (no ACTIVATE -> no 1.3us
ACT_TABLE_LOAD blocking the ACT ring at start).  Real matmuls start
as soon as supertile 0 lands and run cold (1.2 GHz) until HAM
unthrottles ~3.4us after the first warm-up dummy; cold real work
beats idling on dummies.  The last two supertiles are peeled
window-major so the PSUM drain cascades during the final matmuls.
Host pre-scales w^2*X by a power of two into fp8e4 (loss rel-err
~2e-3, tolerance 2e-2), takes column sums for the rank-1 centering
correction, and assembles/reduces in float64.
"""

import numpy as np
import ml_dtypes
from contextlib import ExitStack

import concourse.bass as bass
import concourse.tile as tile
from concourse import bacc, mybir
from concourse import bass_utils

N = 4096
KT = 16                      # k super-tiles of 256 rows (DoubleRow)
UNITS = 10                   # units (128-col half-chunks) per core
ROW = UNITS * 128            # 1280 data cols
WARM_MMS = 7                 # wide dummy matmuls to pre-warm the PE (HAM)
WARM_MMS_NARROW = 6          # fine-grained tail of the warm-up ramp
PEEL = 4                     # trailing super-tiles run window-major so the
                             # PSUM drains cascade under the final matmuls

# Parent-level K2,3 cover of K10: core c loads left parents L (slots
# 0-3) and right parents R (slots 4-9); covers all L x R unit pairs.
COVER = [
    ((5, 7), (6, 4, 3)),
    ((7, 6), (2, 0, 8)),
    ((4, 5), (7, 1, 2)),
    ((6, 8), (4, 3, 5)),
    ((1, 9), (0, 6, 7)),
    ((8, 3), (4, 0, 1)),
    ((9, 0), (4, 2, 5)),
    ((9, 2), (3, 8, 1)),
]
ASSIGN = [
    [2 * p + h for p in L + R for h in (0, 1)] for (L, R) in COVER
]

# (stat_slot, moving_start_col, n_cols): slots 0-3 against slots 4-7,
# slots 8 and 9 against slots 0-3  ->  full K4,6 = 24 blocks.
WINDOWS = [
    (0, 512, 512),
    (1, 512, 512),
    (2, 512, 512),
    (3, 512, 512),
    (8, 0, 512),
    (9, 0, 512),
]
OUT_COLS = sum(w[2] for w in WINDOWS)   # 3072

_CACHE = {}


def _build(c_out):
    f32 = mybir.dt.float32
    f8 = mybir.dt.float8e4
    DR = mybir.MatmulPerfMode.DoubleRow
    nc = bacc.Bacc("TRN2", target_bir_lowering=False, debug=False,
                   num_devices=8)
    # input layout [half, partition, supertile, col]: h-major so one DMA
    # can carry one half of SEVERAL supertiles as a single 3D transfer.
    x = nc.dram_tensor("x", [2, 128, KT, ROW], f8, kind="ExternalInput").ap()
    out = nc.dram_tensor("out", [128, OUT_COLS], f8,
                         kind="ExternalOutput").ap()

    with tile.TileContext(nc) as tc:
        with ExitStack() as ctx:
            zpool = ctx.enter_context(tc.tile_pool(name="z", bufs=1))
            xpool = ctx.enter_context(tc.tile_pool(name="xs", bufs=1))
            psum = ctx.enter_context(tc.tile_pool(name="ps", bufs=1,
                                                  space="PSUM"))
            opool = ctx.enter_context(tc.tile_pool(name="o", bufs=1))

            ps = []
            for i, (_, _, nw) in enumerate(WINDOWS):
                pst = psum.tile([128, nw], f32, tag=f"ps{i}", name=f"ps{i}")
                ps.append(pst)

            # PE warm-up: dummy DoubleRow matmuls start the HAM activity
            # window (~3.4us to unthrottle 1.2 -> 2.4 GHz).  The input
            # DMA completion semaphores fire only ~2.2-2.6us after the
            # transfer ends (~10.2-10.5us), so the ramp bridges from
            # body entry until then.  Products land in ps[0] and are
            # discarded (the real start=True matmul resets it).
            zt = zpool.tile([128, 2, 128], f8, tag="zt")
            nc.vector.memset(zt[:], 0.0)
            zr = zt[:, :, :].unsqueeze(2).broadcast_to([128, 2, 4, 128])
            for _ in range(WARM_MMS):
                nc.tensor.matmul(ps[0][:, 0:512], zt[:, :, :], zr,
                                 start=True, stop=True, perf_mode=DR)
            for _ in range(WARM_MMS_NARROW):
                nc.tensor.matmul(ps[0][:, 0:128], zt[:, :, :], zt[:, :, :],
                                 start=True, stop=True, perf_mode=DR)

            # Input: halves ride the two concurrent HWDGE rings (sync +
            # scalar).  Super-tiles 0 and 1 go as single transfers for
            # the lowest first-tile latency; the rest are batched two
            # supertiles per DMA (328 KB) to amortize the ~2-4us
            # completion-semaphore latency across the 4 in-flight lanes
            # per ring (unbatched, the lanes run dry and the feed
            # stalls).  fb is h-major [128, half, supertile, col] so a
            # batch is one strided 3D transfer.
            fb = xpool.tile([128, 2, KT, ROW], f8, name="fb")
            for k in (0, 1):
                nc.sync.dma_start(fb[:, 0, k, :], x[0, :, k, :])
                nc.scalar.dma_start(fb[:, 1, k, :], x[1, :, k, :])
            for b in range(2, KT, 2):
                nc.sync.dma_start(fb[:, 0, b:b + 2, :], x[0, :, b:b + 2, :])
                nc.scalar.dma_start(fb[:, 1, b:b + 2, :], x[1, :, b:b + 2, :])
            for k in range(KT - PEEL):
                for wi, (s, mc, nw) in enumerate(WINDOWS):
                    nc.tensor.matmul(
                        ps[wi][:, 0:nw],
                        fb[:, :, k, s * 128:(s + 1) * 128],
                        fb[:, :, k, mc:mc + nw],
                        start=(k == 0),
                        stop=False,
                        perf_mode=DR,
                    )
            for wi, (s, mc, nw) in enumerate(WINDOWS):
                for k in range(KT - PEEL, KT):
                    nc.tensor.matmul(
                        ps[wi][:, 0:nw],
                        fb[:, :, k, s * 128:(s + 1) * 128],
                        fb[:, :, k, mc:mc + nw],
                        start=False,
                        stop=(k == KT - 1),
                        perf_mode=DR,
                    )

            # PSUM -> SBUF on vector only, scaled into fp8 range (c_out
            # is a power of two picked on the host so |A|*c_out <= 224
            # by Cauchy-Schwarz); three output DMAs so transfers overlap
            # the remaining copies.
            ot = opool.tile([128, OUT_COLS], f8)
            col = 0
            cuts = []
            for wi, (s, mc, nw) in enumerate(WINDOWS):
                nc.vector.tensor_scalar_mul(ot[:, col:col + nw],
                                            ps[wi][:, 0:nw], c_out)
                col += nw
                if wi in (1, 3, 4, 5):
                    cuts.append(col)
            lo = 0
            for hi in cuts:
                nc.sync.dma_start(out[:, lo:hi], ot[:, lo:hi])
                lo = hi
    nc.compile()
    return nc


def _get_nc(c_out):
    if _CACHE.get("c_out") != c_out:
        _CACHE["nc"] = _build(c_out)
        _CACHE["c_out"] = c_out
    return _CACHE["nc"]


def _quantize(X, w):
    """Host prep: F = w^2 * X, scaled by a power of two into fp8e4 range."""
    F = (w.astype(np.float64) ** 2) * X.astype(np.float64)
    amax = float(np.abs(F).max())
    if amax == 0.0 or not np.isfinite(amax):
        scale = 1.0
    else:
        scale = 2.0 ** np.floor(np.log2(192.0 / amax))
    Fq = np.clip(F * scale, -240.0, 240.0).astype(ml_dtypes.float8_e4m3)
    return Fq, scale


def _in_maps(Fq):
    maps = []
    for units in ASSIGN:
        xc = np.concatenate([Fq[:, u * 128:(u + 1) * 128] for u in units],
                            axis=1)
        # device layout [half, partition, supertile, col]
        xd = np.ascontiguousarray(
            xc.reshape(KT, 2, 128, ROW).transpose(1, 2, 0, 3))
        maps.append({"x": xd})
    return maps


def _assemble(outs, svec, scale, c_out):
    inv = 1.0 / (scale * scale * c_out)
    quad = {}
    for c, units in enumerate(ASSIGN):
        o = outs[c].astype(np.float64) * inv
        col = 0
        for (s, mc, nw) in WINDOWS:
            su = units[s]
            block = o[:, col:col + nw]
            col += nw
            m0 = mc // 128
            for t in range(nw // 128):
                quad[(su, units[m0 + t])] = block[:, t * 128:(t + 1) * 128]
    loss = 0.0
    for i in range(10):
        s_i = np.concatenate([svec[2 * i], svec[2 * i + 1]])
        for j in range(i + 1, 10):
            s_j = np.concatenate([svec[2 * j], svec[2 * j + 1]])
            A = np.empty((256, 256))
            for a in range(2):
                for b in range(2):
                    u, v = 2 * i + a, 2 * j + b
                    q = quad[(u, v)] if (u, v) in quad else quad[(v, u)].T
                    A[a * 128:(a + 1) * 128, b * 128:(b + 1) * 128] = q
            C = A - np.outer(s_i, s_j) / float(N)
            loss += float((C * C).sum())
    loss /= float((N - 1) * (N - 1))
    return np.asarray([loss], np.float32)


def kernel(final_readout, weight, _trace=False):
    X = np.ascontiguousarray(np.asarray(final_readout, np.float32))
    w = np.asarray(weight, np.float32)
    Fq, scale = _quantize(X, w)
    # column sums of the quantized data (exact, fp64) for the centering
    # correction; must match the data the device saw.
    Fq64 = Fq.astype(np.float64)
    scol = Fq64.sum(axis=0) / scale
    svec = {u: scol[u * 128:(u + 1) * 128] for u in range(20)}
    # output rescale: |A_q| <= max col norm squared (Cauchy-Schwarz), so
    # A_q * c_out fits fp8e4 range with no clipping
    cn2 = float((Fq64 * Fq64).sum(axis=0).max())
    c_out = 2.0 ** np.floor(np.log2(224.0 / cn2)) if cn2 > 0 else 1.0
    nc = _get_nc(c_out)
    res = bass_utils.run_bass_kernel_spmd(
        nc, _in_maps(Fq), core_ids=list(range(8)), trace=_trace)
    _CACHE["last_results"] = res
    return _assemble([r["out"] for r in res.results], svec, scale, c_out)


# revision 19
# speedup vs baseline: 1.0334x; 1.0334x over previous
"""HSIC pairwise loss kernel for trn2 (8 NeuronCores), fp8 DoubleRow.

Math: reference builds K_c = (w^2 w^2T) * (E_c E_c^T), M_c = R K_c, and sums
tr(M_i M_j) over i<j. With F_c = w^2 * E_c (row scaling), R the centering
matrix (idempotent):
    tr(R K_i R K_j) = ||G_i^T G_j||_F^2,  G_c = F_c - colmean(F_c)
and with A_ij = F_i^T F_j, s_c = F_c^T 1:
    G_i^T G_j = A_ij - (1/n) s_i s_j^T
so loss = sum_{i<j} ||A_ij - s_i s_j^T / n||_F^2 / (n-1)^2.

Device work: the 45 A_ij blocks [256,256] (contraction over n=4096) at
half-chunk granularity: 180 unordered cross-parent pairs of the 20
128-col units.  Decomposition: at the PARENT (chunk) level, cover K10's
45 edges with 8 bipartite K2,3 graphs (one per core, exactly 3 edges
double-covered).  Core c loads its 2 left parents (4 units, slots 0-3)
and 3 right parents (6 units, slots 4-9) = 10 units, 5.2 MB fp8, and
computes all 24 left x right unit blocks as six 512-wide DoubleRow
windows into 6 PSUM banks (24 blocks/core vs the 26 of the previous
window-search assignment; 22.5 is the absolute lower bound).

Input DMA rides BOTH HWDGE rings (sync/SP + scalar/ACT, concurrent
FIFO rings) - supertile halves split across rings, so supertile 0
lands ~0.7us after trigger and the stream sustains ~2x one ring.  The
PSUM drain runs on vector only (no ACTIVATE -> no 1.3us
ACT_TABLE_LOAD blocking the ACT ring at start).  Real matmuls start
as soon as supertile 0 lands and run cold (1.2 GHz) until HAM
unthrottles ~3.4us after the first warm-up dummy; cold real work
beats idling on dummies.  The last two supertiles are peeled
window-major so the PSUM drain cascades during the final matmuls.
Host pre-scales w^2*X by a power of two into fp8e4 (loss rel-err
~2e-3, tolerance 2e-2), takes column sums for the rank-1 centering
correction, and assembles/reduces in float64.
"""

import numpy as np
import ml_dtypes
from contextlib import ExitStack

import concourse.bass as bass
import concourse.tile as tile
from concourse import bacc, mybir
from concourse import bass_utils

N = 4096
KT = 16                      # k super-tiles of 256 rows (DoubleRow)
UNITS = 10                   # units (128-col half-chunks) per core
ROW = UNITS * 128            # 1280 data cols
WARM_MMS = 7                 # wide dummy matmuls to pre-warm the PE (HAM)
WARM_MMS_NARROW = 6          # fine-grained tail of the warm-up ramp
PEEL = 4                     # trailing super-tiles run window-major so the
                             # PSUM drains cascade under the final matmuls

# Parent-level K2,3 cover of K10: core c loads left parents L (slots
# 0-3) and right parents R (slots 4-9); covers all L x R unit pairs.
COVER = [
    ((5, 7), (6, 4, 3)),
    ((7, 6), (2, 0, 8)),
    ((4, 5), (7, 1, 2)),
    ((6, 8), (4, 3, 5)),
    ((1, 9), (0, 6, 7)),
    ((8, 3), (4, 0, 1)),
    ((9, 0), (4, 2, 5)),
    ((9, 2), (3, 8, 1)),
]
ASSIGN = [
    [2 * p + h for p in L + R for h in (0, 1)] for (L, R) in COVER
]

# (stat_slot, moving_start_col, n_cols): slots 0-3 against slots 4-7,
# slots 8 and 9 against slots 0-3  ->  full K4,6 = 24 blocks.
WINDOWS = [
    (0, 512, 512),
    (1, 512, 512),
    (2, 512, 512),
    (3, 512, 512),
    (8, 0, 512),
    (9, 0, 512),
]
OUT_COLS = sum(w[2] for w in WINDOWS)   # 3072

_CACHE = {}


def _build(c_out):
    f32 = mybir.dt.float32
    f8 = mybir.dt.float8e4
    DR = mybir.MatmulPerfMode.DoubleRow
    nc = bacc.Bacc("TRN2", target_bir_lowering=False, debug=False,
                   num_devices=8)
    x = nc.dram_tensor("x", [N, ROW], f8, kind="ExternalInput").ap()
    out = nc.dram_tensor("out", [128, OUT_COLS], f8,
                         kind="ExternalOutput").ap()

    with tile.TileContext(nc) as tc:
        with ExitStack() as ctx:
            zpool = ctx.enter_context(tc.tile_pool(name="z", bufs=1))
            xpool = ctx.enter_context(tc.tile_pool(name="xs", bufs=1))
            psum = ctx.enter_context(tc.tile_pool(name="ps", bufs=1,
                                                  space="PSUM"))
            opool = ctx.enter_context(tc.tile_pool(name="o", bufs=1))

            ps = []
            for i, (_, _, nw) in enumerate(WINDOWS):
                pst = psum.tile([128, nw], f32, tag=f"ps{i}", name=f"ps{i}")
                ps.append(pst)

            # PE warm-up: dummy DoubleRow matmuls start the HAM activity
            # window (~3.4us to unthrottle 1.2 -> 2.4 GHz).  The input
            # DMA completion semaphores fire only ~2.2-2.6us after the
            # transfer ends (~10.2-10.5us), so the ramp bridges from
            # body entry until then.  Products land in ps[0] and are
            # discarded (the real start=True matmul resets it).
            zt = zpool.tile([128, 2, 128], f8, tag="zt")
            nc.vector.memset(zt[:], 0.0)
            zr = zt[:, :, :].unsqueeze(2).broadcast_to([128, 2, 4, 128])
            for _ in range(WARM_MMS):
                nc.tensor.matmul(ps[0][:, 0:512], zt[:, :, :], zr,
                                 start=True, stop=True, perf_mode=DR)
            for _ in range(WARM_MMS_NARROW):
                nc.tensor.matmul(ps[0][:, 0:128], zt[:, :, :], zt[:, :, :],
                                 start=True, stop=True, perf_mode=DR)

            # Input: supertile halves on the two concurrent HWDGE rings
            # (sync + scalar).  Super-tiles 0..KT-PEEL-1 supertile-major;
            # the last PEEL peeled window-major below so the PSUM drains
            # cascade DURING the final matmuls.
            fts = []
            for k in range(KT):
                ft = xpool.tile([128, 2, ROW], f8, name=f"ft{k}")
                nc.sync.dma_start(ft[:, 0, :], x[k * 256:k * 256 + 128, :])
                nc.scalar.dma_start(ft[:, 1, :],
                                    x[k * 256 + 128:k * 256 + 256, :])
                fts.append(ft)
                if k >= KT - PEEL:
                    continue
                for wi, (s, mc, nw) in enumerate(WINDOWS):
                    nc.tensor.matmul(
                        ps[wi][:, 0:nw],
                        ft[:, :, s * 128:(s + 1) * 128],
                        ft[:, :, mc:mc + nw],
                        start=(k == 0),
                        stop=False,
                        perf_mode=DR,
                    )
            for wi, (s, mc, nw) in enumerate(WINDOWS):
                for k in range(KT - PEEL, KT):
                    ft = fts[k]
                    nc.tensor.matmul(
                        ps[wi][:, 0:nw],
                        ft[:, :, s * 128:(s + 1) * 128],
                        ft[:, :, mc:mc + nw],
                        start=False,
                        stop=(k == KT - 1),
                        perf_mode=DR,
                    )

            # PSUM -> SBUF on vector only, scaled into fp8 range (c_out
            # is a power of two picked on the host so |A|*c_out <= 224
            # by Cauchy-Schwarz); three output DMAs so transfers overlap
            # the remaining copies.
            ot = opool.tile([128, OUT_COLS], f8)
            col = 0
            cuts = []
            for wi, (s, mc, nw) in enumerate(WINDOWS):
                nc.vector.tensor_scalar_mul(ot[:, col:col + nw],
                                            ps[wi][:, 0:nw], c_out)
                col += nw
                if wi in (1, 3, 4, 5):
                    cuts.append(col)
            lo = 0
            for hi in cuts:
                nc.sync.dma_start(out[:, lo:hi], ot[:, lo:hi])
                lo = hi
    nc.compile()
    return nc


def _get_nc(c_out):
    if _CACHE.get("c_out") != c_out:
        _CACHE["nc"] = _build(c_out)
        _CACHE["c_out"] = c_out
    return _CACHE["nc"]


def _quantize(X, w):
    """Host prep: F = w^2 * X, scaled by a power of two into fp8e4 range."""
    F = (w.astype(np.float64) ** 2) * X.astype(np.float64)
    amax = float(np.abs(F).max())
    if amax == 0.0 or not np.isfinite(amax):
        scale = 1.0
    else:
        scale = 2.0 ** np.floor(np.log2(192.0 / amax))
    Fq = np.clip(F * scale, -240.0, 240.0).astype(ml_dtypes.float8_e4m3)
    return Fq, scale


def _in_maps(Fq):
    maps = []
    for units in ASSIGN:
        xc = np.concatenate([Fq[:, u * 128:(u + 1) * 128] for u in units],
                            axis=1)
        maps.append({"x": np.ascontiguousarray(xc)})
    return maps


def _assemble(outs, svec, scale, c_out):
    inv = 1.0 / (scale * scale * c_out)
    quad = {}
    for c, units in enumerate(ASSIGN):
        o = outs[c].astype(np.float64) * inv
        col = 0
        for (s, mc, nw) in WINDOWS:
            su = units[s]
            block = o[:, col:col + nw]
            col += nw
            m0 = mc // 128
            for t in range(nw // 128):
                quad[(su, units[m0 + t])] = block[:, t * 128:(t + 1) * 128]
    loss = 0.0
    for i in range(10):
        s_i = np.concatenate([svec[2 * i], svec[2 * i + 1]])
        for j in range(i + 1, 10):
            s_j = np.concatenate([svec[2 * j], svec[2 * j + 1]])
            A = np.empty((256, 256))
            for a in range(2):
                for b in range(2):
                    u, v = 2 * i + a, 2 * j + b
                    q = quad[(u, v)] if (u, v) in quad else quad[(v, u)].T
                    A[a * 128:(a + 1) * 128, b * 128:(b + 1) * 128] = q
            C = A - np.outer(s_i, s_j) / float(N)
            loss += float((C * C).sum())
    loss /= float((N - 1) * (N - 1))
    return np.asarray([loss], np.float32)


def kernel(final_readout, weight, _trace=False):
    X = np.ascontiguousarray(np.asarray(final_readout, np.float32))
    w = np.asarray(weight, np.float32)
    Fq, scale = _quantize(X, w)
    # column sums of the quantized data (exact, fp64) for the centering
    # correction; must match the data the device saw.
    Fq64 = Fq.astype(np.float64)
    scol = Fq64.sum(axis=0) / scale
    svec = {u: scol[u * 128:(u + 1) * 128] for u in range(20)}
    # output rescale: |A_q| <= max col norm squared (Cauchy-Schwarz), so
    # A_q * c_out fits fp8e4 range with no clipping
    cn2 = float((Fq64 * Fq64).sum(axis=0).max())
    c_out = 2.0 ** np.floor(np.log2(224.0 / cn2)) if cn2 > 0 else 1.0
    nc = _get_nc(c_out)
    res = bass_utils.run_bass_kernel_spmd(
        nc, _in_maps(Fq), core_ids=list(range(8)), trace=_trace)
    _CACHE["last_results"] = res
    return _assemble([r["out"] for r in res.results], svec, scale, c_out)


# revision 22
# speedup vs baseline: 1.0470x; 1.0131x over previous
"""HSIC pairwise loss kernel for trn2 (8 NeuronCores), fp8 DoubleRow.

Math: reference builds K_c = (w^2 w^2T) * (E_c E_c^T), M_c = R K_c, and sums
tr(M_i M_j) over i<j. With F_c = w^2 * E_c (row scaling), R the centering
matrix (idempotent):
    tr(R K_i R K_j) = ||G_i^T G_j||_F^2,  G_c = F_c - colmean(F_c)
and with A_ij = F_i^T F_j, s_c = F_c^T 1:
    G_i^T G_j = A_ij - (1/n) s_i s_j^T
so loss = sum_{i<j} ||A_ij - s_i s_j^T / n||_F^2 / (n-1)^2.

Device work: the 45 A_ij blocks [256,256] (contraction over n=4096) at
half-chunk granularity: 180 unordered cross-parent pairs of the 20
128-col units.  Decomposition: at the PARENT (chunk) level, cover K10's
45 edges with 8 bipartite K2,3 graphs (one per core, exactly 3 edges
double-covered).  Core c loads its 2 left parents (4 units, slots 0-3)
and 3 right parents (6 units, slots 4-9) = 10 units, 5.2 MB fp8, and
computes all 24 left x right unit blocks as six 512-wide DoubleRow
windows into 6 PSUM banks (24 blocks/core vs the 26 of the previous
window-search assignment; 22.5 is the absolute lower bound).

Input DMA rides BOTH HWDGE rings (sync/SP + scalar/ACT, concurrent
FIFO rings) - supertile halves split across rings, so supertile 0
lands ~0.7us after trigger and the stream sustains ~2x one ring.  The
PSUM drain runs on vector only (no ACTIVATE -> no 1.3us
ACT_TABLE_LOAD blocking the ACT ring at start).  Real matmuls start
as soon as supertile 0 lands and run cold (1.2 GHz) until HAM
unthrottles ~3.4us after the first warm-up dummy; cold real work
beats idling on dummies.  The last two supertiles are peeled
window-major so the PSUM drain cascades during the final matmuls.
Host pre-scales w^2*X by a power of two into fp8e4 (loss rel-err
~2e-3, tolerance 2e-2), takes column sums for the rank-1 centering
correction, and assembles/reduces in float64.
"""

import numpy as np
import ml_dtypes
from contextlib import ExitStack

import concourse.bass as bass
import concourse.tile as tile
from concourse import bacc, mybir
from concourse import bass_utils

N = 4096
KT = 16                      # k super-tiles of 256 rows (DoubleRow)
UNITS = 10                   # units (128-col half-chunks) per core
ROW = UNITS * 128            # 1280 data cols
WARM_MMS = 7                 # wide dummy matmuls to pre-warm the PE (HAM)
WARM_MMS_NARROW = 6          # fine-grained tail of the warm-up ramp
PEEL = 4                     # trailing super-tiles run window-major so the
                             # PSUM drains cascade under the final matmuls

# Parent-level K2,3 cover of K10: core c loads left parents L (slots
# 0-3) and right parents R (slots 4-9); covers all L x R unit pairs.
COVER = [
    ((5, 7), (6, 4, 3)),
    ((7, 6), (2, 0, 8)),
    ((4, 5), (7, 1, 2)),
    ((6, 8), (4, 3, 5)),
    ((1, 9), (0, 6, 7)),
    ((8, 3), (4, 0, 1)),
    ((9, 0), (4, 2, 5)),
    ((9, 2), (3, 8, 1)),
]
ASSIGN = [
    [2 * p + h for p in L + R for h in (0, 1)] for (L, R) in COVER
]

# (stat_slot, moving_start_col, n_cols): slots 0-3 against slots 4-7,
# slots 8 and 9 against slots 0-3  ->  full K4,6 = 24 blocks.
WINDOWS = [
    (0, 512, 512),
    (1, 512, 512),
    (2, 512, 512),
    (3, 512, 512),
    (8, 0, 512),
    (9, 0, 512),
]
OUT_COLS = sum(w[2] for w in WINDOWS)   # 3072

_CACHE = {}


def _build(c_out):
    f32 = mybir.dt.float32
    f8 = mybir.dt.float8e4
    DR = mybir.MatmulPerfMode.DoubleRow
    nc = bacc.Bacc("TRN2", target_bir_lowering=False, debug=False,
                   num_devices=8)
    # input layout [half, partition, supertile, col]: lets one DMA carry
    # one half of several supertiles as a single strided 3D transfer.
    x = nc.dram_tensor("x", [2, 128, KT, ROW], f8, kind="ExternalInput").ap()
    out = nc.dram_tensor("out", [128, OUT_COLS], f8,
                         kind="ExternalOutput").ap()

    with tile.TileContext(nc) as tc:
        with ExitStack() as ctx:
            zpool = ctx.enter_context(tc.tile_pool(name="z", bufs=1))
            xpool = ctx.enter_context(tc.tile_pool(name="xs", bufs=1))
            psum = ctx.enter_context(tc.tile_pool(name="ps", bufs=1,
                                                  space="PSUM"))
            opool = ctx.enter_context(tc.tile_pool(name="o", bufs=1))

            ps = []
            for i, (_, _, nw) in enumerate(WINDOWS):
                pst = psum.tile([128, nw], f32, tag=f"ps{i}", name=f"ps{i}")
                ps.append(pst)

            # PE warm-up: dummy DoubleRow matmuls start the HAM activity
            # window (~3.4us to unthrottle 1.2 -> 2.4 GHz).  The input
            # DMA completion semaphores fire only ~2.2-2.6us after the
            # transfer ends (~10.2-10.5us), so the ramp bridges from
            # body entry until then.  Products land in ps[0] and are
            # discarded (the real start=True matmul resets it).
            zt = zpool.tile([128, 2, 128], f8, tag="zt")
            nc.vector.memset(zt[:], 0.0)
            zr = zt[:, :, :].unsqueeze(2).broadcast_to([128, 2, 4, 128])
            for _ in range(WARM_MMS):
                nc.tensor.matmul(ps[0][:, 0:512], zt[:, :, :], zr,
                                 start=True, stop=True, perf_mode=DR)
            for _ in range(WARM_MMS_NARROW):
                nc.tensor.matmul(ps[0][:, 0:128], zt[:, :, :], zt[:, :, :],
                                 start=True, stop=True, perf_mode=DR)

            # Input: supertile halves on the two concurrent HWDGE rings
            # (sync h0 + scalar h1).  Super-tiles 0 and 1 ride ONE
            # transfer per ring (their completion semaphores fire
            # ~2.5-4us after the transfer ends, and a ring's 2nd
            # transfer completes ~1.9us after its 1st -- batching makes
            # BOTH supertiles consumable right as the warm-up ramp ends
            # instead of stalling the PE after supertile 0).  k>=2 go as
            # singles so the matmul wait granularity stays fine.
            f01 = xpool.tile([128, 2, 2, ROW], f8, name="f01")
            nc.sync.dma_start(f01[:, 0, :, :], x[0, :, 0:2, :])
            nc.scalar.dma_start(f01[:, 1, :, :], x[1, :, 0:2, :])
            fts = [f01[:, :, 0, :], f01[:, :, 1, :]]
            for k in range(2, KT):
                ft = xpool.tile([128, 2, ROW], f8, name=f"ft{k}")
                nc.sync.dma_start(ft[:, 0, :], x[0, :, k, :])
                nc.scalar.dma_start(ft[:, 1, :], x[1, :, k, :])
                fts.append(ft[:, :, :])
            for k in range(KT - PEEL):
                ft = fts[k]
                for wi, (s, mc, nw) in enumerate(WINDOWS):
                    nc.tensor.matmul(
                        ps[wi][:, 0:nw],
                        ft[:, :, s * 128:(s + 1) * 128],
                        ft[:, :, mc:mc + nw],
                        start=(k == 0),
                        stop=False,
                        perf_mode=DR,
                    )
            for wi, (s, mc, nw) in enumerate(WINDOWS):
                for k in range(KT - PEEL, KT):
                    ft = fts[k]
                    nc.tensor.matmul(
                        ps[wi][:, 0:nw],
                        ft[:, :, s * 128:(s + 1) * 128],
                        ft[:, :, mc:mc + nw],
                        start=False,
                        stop=(k == KT - 1),
                        perf_mode=DR,
                    )

            # PSUM -> SBUF on vector only, scaled into fp8 range (c_out
            # is a power of two picked on the host so |A|*c_out <= 224
            # by Cauchy-Schwarz); three output DMAs so transfers overlap
            # the remaining copies.
            ot = opool.tile([128, OUT_COLS], f8)
            col = 0
            cuts = []
            for wi, (s, mc, nw) in enumerate(WINDOWS):
                nc.vector.tensor_scalar_mul(ot[:, col:col + nw],
                                            ps[wi][:, 0:nw], c_out)
                col += nw
                if wi in (1, 3, 4, 5):
                    cuts.append(col)
            lo = 0
            for hi in cuts:
                nc.sync.dma_start(out[:, lo:hi], ot[:, lo:hi])
                lo = hi
    nc.compile()
    return nc


def _get_nc(c_out):
    if _CACHE.get("c_out") != c_out:
        _CACHE["nc"] = _build(c_out)
        _CACHE["c_out"] = c_out
    return _CACHE["nc"]


def _quantize(X, w):
    """Host prep: F = w^2 * X, scaled by a power of two into fp8e4 range."""
    F = (w.astype(np.float64) ** 2) * X.astype(np.float64)
    amax = float(np.abs(F).max())
    if amax == 0.0 or not np.isfinite(amax):
        scale = 1.0
    else:
        scale = 2.0 ** np.floor(np.log2(192.0 / amax))
    Fq = np.clip(F * scale, -240.0, 240.0).astype(ml_dtypes.float8_e4m3)
    return Fq, scale


def _in_maps(Fq):
    maps = []
    for units in ASSIGN:
        xc = np.concatenate([Fq[:, u * 128:(u + 1) * 128] for u in units],
                            axis=1)
        # device layout [half, partition, supertile, col]
        xd = np.ascontiguousarray(
            xc.reshape(KT, 2, 128, ROW).transpose(1, 2, 0, 3))
        maps.append({"x": xd})
    return maps


def _assemble(outs, svec, scale, c_out):
    inv = 1.0 / (scale * scale * c_out)
    quad = {}
    for c, units in enumerate(ASSIGN):
        o = outs[c].astype(np.float64) * inv
        col = 0
        for (s, mc, nw) in WINDOWS:
            su = units[s]
            block = o[:, col:col + nw]
            col += nw
            m0 = mc // 128
            for t in range(nw // 128):
                quad[(su, units[m0 + t])] = block[:, t * 128:(t + 1) * 128]
    loss = 0.0
    for i in range(10):
        s_i = np.concatenate([svec[2 * i], svec[2 * i + 1]])
        for j in range(i + 1, 10):
            s_j = np.concatenate([svec[2 * j], svec[2 * j + 1]])
            A = np.empty((256, 256))
            for a in range(2):
                for b in range(2):
                    u, v = 2 * i + a, 2 * j + b
                    q = quad[(u, v)] if (u, v) in quad else quad[(v, u)].T
                    A[a * 128:(a + 1) * 128, b * 128:(b + 1) * 128] = q
            C = A - np.outer(s_i, s_j) / float(N)
            loss += float((C * C).sum())
    loss /= float((N - 1) * (N - 1))
    return np.asarray([loss], np.float32)


def kernel(final_readout, weight, _trace=False):
    X = np.ascontiguousarray(np.asarray(final_readout, np.float32))
    w = np.asarray(weight, np.float32)
    Fq, scale = _quantize(X, w)
    # column sums of the quantized data (exact, fp64) for the centering
    # correction; must match the data the device saw.
    Fq64 = Fq.astype(np.float64)
    scol = Fq64.sum(axis=0) / scale
    svec = {u: scol[u * 128:(u + 1) * 128] for u in range(20)}
    # output rescale: |A_q| <= max col norm squared (Cauchy-Schwarz), so
    # A_q * c_out fits fp8e4 range with no clipping
    cn2 = float((Fq64 * Fq64).sum(axis=0).max())
    c_out = 2.0 ** np.floor(np.log2(224.0 / cn2)) if cn2 > 0 else 1.0
    nc = _get_nc(c_out)
    res = bass_utils.run_bass_kernel_spmd(
        nc, _in_maps(Fq), core_ids=list(range(8)), trace=_trace)
    _CACHE["last_results"] = res
    return _assemble([r["out"] for r in res.results], svec, scale, c_out)
